# revision 55
# baseline (speedup 1.0000x reference)
"""Multi-head causal self-attention on 8 TRN2 NeuronCores.

Sharding: data parallel over batch (2) x tensor parallel over heads (16 -> 4
groups of 4 heads).  Core c handles batch c//4 and heads 4*(c%4) .. 4*(c%4)+3.
Each core computes a partial output-projection (its 4 heads' contribution,
[S, D]); the host sums the 4 partials per batch and adds the biases.
No device collectives needed.

Per-core device program (all matmul compute in bf16, f32 PSUM accumulate):
  P1: QT,KT = (x @ WqT, x @ WkT) produced transposed [e, s]; V produced
      natural [s, e] with a ones column appended per head (softmax
      denominators fall out of the AV matmul).  Q and K each run as TWO
      s-half passes covering all four e-tiles at once (2x[128,1024] sc +
      4x[128,512] acc = exactly 8 PSUM banks), which halves the per-
      d-chunk input-bandwidth demand during the DMA ramp (~225 GB/s vs
      ~335, above what HBM delivers while ramping).  i-block 3's score
      tiles + exps (group (3,0) fully, (3,1) first quarter) are emitted
      between K and V so ACT (otherwise idle in P1) builds an exp
      reservoir before phase 2 starts.
  P2/P3 software-pipelined one group deep, i-blocks processed in
      DESCENDING order (3,2,1,0) so the serial tail after the last exp is
      the SMALLEST i-block's AV work (~1us) + one i-block of output
      projection, instead of the largest.  Emission order per slot:
      scoresT+exp for group g, then AV (+ O transpose + Wo projection) for
      group g-1.  The Tile scheduler then uses the fully-exp'd previous
      group as filler work for the PE while the current group's scores
      wait on ACT, instead of idling.
      O s-tiles are transposed by a regular matmul against the identity
      (moving operand) -- ~4x cheaper than PE transpose-mode, and exact.

DMA: inputs are consolidated into FEW LARGE transfers (weights as single
  [128, dc, cols] 3D-AP transfers) with a hand-ordered two-ring (SP+ACT)
  issue schedule putting the Q-pass critical path (wq first-half, x chunks)
  first.  This keeps the PE fed through the ramp; stalls >3.4us would drop
  the PE HAM clock gate from 2.4 to 1.2 GHz.

PSUM evacuation alternates DVE / ACT (bias-add via activation Identity,
  normalize-mul via activation Copy with scale AP) so neither engine's
  backlog gates the shared PSUM rings.

Host folds: 1/sqrt(dk) into Wq/bq; V-bias contribution = wo @ bv (rows of a
softmax sum to exactly 1) and bo are added on the host.  Output partials are
bf16 (summed in f32 on the host).
"""

import numpy as np
import ml_dtypes
from contextlib import ExitStack

import concourse.bass as bass
import concourse.mybir as mybir
import concourse.tile as tile
from concourse import bacc
from concourse.bass_utils import run_bass_kernel_spmd
from concourse.masks import make_upper_triangular, make_identity

BF16 = ml_dtypes.bfloat16
F32 = mybir.dt.float32
BF = mybir.dt.bfloat16

B = 2
S = 2048
D = 2048
H = 16
DK = 128
NCORES = 8
HPC = 4                  # heads per core
E = HPC * DK             # 512 = output cols per core for q/k/v
P = 128
NDC = D // P             # 16 d-chunks
NST = S // P             # 16 s-tiles
NSB = S // 512           # 4 s/i blocks of 512
DKP = DK + 1             # dk + ones column
N_WARMUP = 46            # dummy matmuls bridging the PE HAM activity window
                         # from engine start (~7.6us) past the first
                         # x-chunk arrival (~11-13us, run-variable under
                         # 8-core HBM contention): the PE clock is then
                         # already at 2.4GHz when real work begins, and a
                         # late DMA roll cannot re-cool it


def _build_nc():
    nc = bacc.Bacc("TRN2", target_bir_lowering=False, debug=False)

    xt = nc.dram_tensor("xt", [D, S], BF, kind="ExternalInput").ap()
    wqt = nc.dram_tensor("wqt", [D, E], BF, kind="ExternalInput").ap()
    wkt = nc.dram_tensor("wkt", [D, E], BF, kind="ExternalInput").ap()
    wvt = nc.dram_tensor("wvt", [D, E], BF, kind="ExternalInput").ap()
    wot = nc.dram_tensor("wot", [E, D], BF, kind="ExternalInput").ap()
    bqd = nc.dram_tensor("bq", [P, HPC], F32, kind="ExternalInput").ap()
    bkd = nc.dram_tensor("bk", [P, HPC], F32, kind="ExternalInput").ap()
    outd = nc.dram_tensor("out", [S, D], BF, kind="ExternalOutput").ap()

    with tile.TileContext(nc) as tc, ExitStack() as ctx:
        # PSUM: sc 2x[128,1024] (4 banks) + acc 4x[128,512] (4) = 8
        pst = ctx.enter_context(tc.tile_pool(name="pst", bufs=2, space="PSUM"))
        persist = ctx.enter_context(tc.tile_pool(name="persist", bufs=1))

        qt_t = [persist.tile([P, S], BF, name=f"qt{h}", tag=f"qt{h}") for h in range(HPC)]
        kt_t = [persist.tile([P, S], BF, name=f"kt{h}", tag=f"kt{h}") for h in range(HPC)]
        v_t = [persist.tile([P, HPC, DKP], BF, name=f"v{j}", tag=f"v{j}") for j in range(NST)]
        tri = persist.tile([P, P], BF, name="tri", tag="tri")
        ident = persist.tile([P, P], BF, name="ident", tag="ident")
        bq_sb = persist.tile([P, HPC], F32, name="bq_sb", tag="bq_sb")
        bk_sb = persist.tile([P, HPC], F32, name="bk_sb", tag="bk_sb")
        # i-block 3 exp tiles live in the persistent pool: they are written
        # during P1 (between K and V) before the phase-2 exp ring opens.
        # (3,0) is fully primed (16 j-tiles); (3,1) gets its first 6
        # (SBUF-budget limited; the rest flow through the phase-2 ring).
        e30_t = [persist.tile([P, 1024], BF, name=f"e30_{j}", tag=f"e30_{j}")
                 for j in range(16)]
        e31_t = [persist.tile([P, 1024], BF, name=f"e31_{j}", tag=f"e31_{j}")
                 for j in range(6)]

        # PE warmup during the input-DMA ramp (results are never read); the
        # operand is produced by a single fast DVE memset, not gpsimd.
        wupd = persist.tile([P, P], BF, name="wupd", tag="wupd")
        nc.vector.memset(wupd[:], 0.0)
        for i in range(N_WARMUP):
            pw = pst.tile([P, 512], F32, name="pw", tag="acc", bufs=4)
            nc.tensor.matmul(pw[:, 0:P], wupd[:], wupd[:], start=True, stop=True)
        # preload the ACT Exp function table now, off the first-score path
        dexp = persist.tile([P, 1], F32, name="dexp", tag="dexp")
        nc.scalar.activation(dexp[:], wupd[:, 0:1],
                             mybir.ActivationFunctionType.Exp)

        # tri[p, f] = 1.0 iff p <= f  (keep j <= i on the diagonal block)
        make_upper_triangular(nc, tri[:], val=1.0, diag=True)
        make_identity(nc, ident[:])
        for j in range(NST):
            nc.vector.memset(v_t[j][:, :, DK:DKP], 1.0)

        def sc_group(ib, hp, et_alloc, jt_lo=0, jt_hi=None):
            """ScoresT + exp for head-pair hp of i-block ib, j-tiles
            [jt_lo, jt_hi); returns exp tiles."""
            njt = 4 * ib + 4
            if jt_hi is None:
                jt_hi = njt
            etiles = []
            for jt in range(jt_lo, jt_hi):
                pss = pst.tile([P, 1024], F32, name="pss", tag="sc", bufs=2)
                # band tiles only need i >= jt*128: slice N accordingly
                c0 = max(0, (jt - 4 * ib)) * P
                for k in range(2):
                    h = 2 * hp + k
                    nc.tensor.matmul(
                        pss[:, k * 512 + c0:(k + 1) * 512],
                        kt_t[h][:, jt * P:(jt + 1) * P],
                        qt_t[h][:, ib * 512 + c0:(ib + 1) * 512],
                        start=True, stop=True)
                et_t = et_alloc(jt)
                if jt <= 4 * ib:
                    # full tile written by the matmuls above: one 2D exp
                    nc.scalar.activation(
                        et_t[:], pss[:], mybir.ActivationFunctionType.Exp)
                else:
                    # diag tile: exp only the written per-head regions
                    # (strided 3D AP keeps the read inside this tenant's
                    # writes -- avoids stale-PSUM reads)
                    s_off = jt - 4 * ib
                    et3 = et_t[:].rearrange("p (h w) -> p h w", h=2)
                    ps3 = pss[:].rearrange("p (h w) -> p h w", h=2)
                    nc.scalar.activation(
                        et3[:, :, s_off * P:512], ps3[:, :, s_off * P:512],
                        mybir.ActivationFunctionType.Exp)
                if jt >= 4 * ib:
                    s_off = jt - 4 * ib
                    # zero the diag-masked part of both heads at once
                    et3 = et_t[:].rearrange("p (h w) -> p h w", h=2)
                    nc.vector.tensor_tensor(
                        et3[:, :, s_off * P:(s_off + 1) * P],
                        et3[:, :, s_off * P:(s_off + 1) * P],
                        tri[:, None, :].to_broadcast([P, 2, P]),
                        mybir.AluOpType.mult)
                etiles.append(et_t)
            return etiles

        group_etiles = {}

        # ------------------------------------------------------------------
        # Phase 1: QT/KT [e, s] and V [s, e]; i-block 3 scores between K and V
        # ------------------------------------------------------------------
        with tc.tile_pool(name="pxv", bufs=1) as pxv:
            xt_t = [pxv.tile([P, S], BF, name=f"xt{dc}", tag=f"xt{dc}") for dc in range(NDC)]
            # wk/wv live in the P1-wide pool: their SBUF must NOT overlap
            # the wq pool, or their DMAs would wait for Q to finish reading
            # wq (address-range reuse dependency) -- that was the baseline's
            # 3.3us K-phase stall (and the HAM re-cool it triggered).
            wk_sb = pxv.tile([P, NDC, E], BF, name="wk_sb", tag="wk_sb")
            wv_sb = pxv.tile([P, NDC, E], BF, name="wv_sb", tag="wv_sb")

            # ---- DMA issue plan -------------------------------------------
            # Few LARGE transfers, priority-ordered on the two HWDGE rings
            # (SP + ACT).  Q/K run in TWO s-half passes with all four
            # e-tiles at once (psum: 2x[128,1024] sc + 4x[128,512] acc =
            # exactly 8 banks), so pass A consumes only each x chunk's
            # first half: per-dc demand is wq 128KB + xt 256KB per 1.7us
            # (~225 GB/s) instead of 576KB (~335 GB/s, above what HBM
            # delivers during the ramp).  Each dma_start costs ~0.75us of
            # serial issue time on its ring, so weights move as single
            # 3D-AP [p, dc, cols] transfers.
            with tc.tile_pool(name="pq", bufs=1) as pq:
                wq_sb = pq.tile([P, NDC, E], BF, name="wq_sb", tag="wq_sb")

                def wq_piece(lo, hi, eng=None):
                    eng = eng or nc.sync
                    if lo + 1 == hi:
                        eng.dma_start(wq_sb[:, lo, :], wqt[lo * P:hi * P, :])
                    else:
                        eng.dma_start(
                            wq_sb[:, lo:hi, :],
                            wqt[lo * P:hi * P, :].rearrange(
                                "(dc p) e -> p dc e", p=P))

                # pass-A pieces (xt first halves) in consumption order,
                # alternating rings per dc; wq piece for a dc group goes
                # just before that group's x chunks
                # wq dc0 lands in two halves, e-tiles 2/3 first (the pass
                # consumes ets in (2,3,0,1) order): the first real matmul
                # only needs 64KB of weights + the first x piece
                nc.sync.dma_start(wq_sb[:, 0, 256:512], wqt[0:P, 256:512])
                nc.sync.dma_start(xt_t[0][:, 0:512], xt[0:P, 0:512])
                nc.sync.dma_start(wq_sb[:, 0, 0:256], wqt[0:P, 0:256])
                nc.scalar.dma_start(xt_t[0][:, 512:1024], xt[0:P, 512:1024])
                wq_piece(1, 4, nc.scalar)
                for dc in range(1, NDC):
                    if dc == 4:
                        wq_piece(4, 8, nc.scalar)
                    elif dc == 8:
                        wq_piece(8, 12)
                    elif dc == 12:
                        wq_piece(12, 16, nc.scalar)
                    eng = nc.sync if dc % 2 == 0 else nc.scalar
                    eng.dma_start(xt_t[dc][:, 0:1024],
                                  xt[dc * P:(dc + 1) * P, 0:1024])
                # pass-B pieces (xt second halves), then the late weights
                for dc in range(NDC):
                    eng = nc.sync if dc % 2 == 0 else nc.scalar
                    eng.dma_start(xt_t[dc][:, 1024:2048],
                                  xt[dc * P:(dc + 1) * P, 1024:2048])
                # bq/bk ride SWDGE (idle gpsimd): tiny transfers that must
                # land before the pass-A evacs (~44us) without displacing
                # x-chunk issue slots on the HWDGE rings.  wk/wv go on the
                # SP ring, which has NO compute behind it in P1 -- a DMA
                # issue hitting ring backpressure blocks every later
                # instruction on its engine's FIFO (on ACT that would
                # block the evacs).
                nc.gpsimd.dma_start(bq_sb[:], bqd[:, :])
                nc.gpsimd.dma_start(bk_sb[:], bkd[:, :])
                nc.sync.dma_start(
                    wk_sb[:], wkt.rearrange("(dc p) e -> p dc e", p=P))
                nc.sync.dma_start(
                    wv_sb[:], wvt.rearrange("(dc p) e -> p dc e", p=P))

                def qk_pass(wsrc, dest, bias_sb, sbp):
                    """One s-half pass over all 4 e-tiles.  e-tiles 0/1
                    accumulate in the two [128,1024] sc banks, 2/3 in the
                    four [128,512] acc banks."""
                    psA = {et: pst.tile([P, 1024], F32, name=f"psA{et}",
                                        tag="sc", bufs=2) for et in (0, 1)}
                    psB = {et: [pst.tile([P, 512], F32, name=f"psB{et}_{i}",
                                         tag="acc", bufs=4) for i in range(2)]
                           for et in (2, 3)}
                    def evac_et(et):
                        use_act = (et % 2 == 1)
                        srcs = ([psA[et][:]] if et < 2
                                else [psB[et][0][:], psB[et][1][:]])
                        off = sbp * 1024
                        for src in srcs:
                            w = src.shape[-1]
                            dst = dest[et][:, off:off + w]
                            off += w
                            if use_act:
                                nc.scalar.activation(
                                    dst, src,
                                    mybir.ActivationFunctionType.Identity,
                                    bias=bias_sb[:, et:et + 1])
                            else:
                                nc.vector.tensor_scalar_add(
                                    dst, src, bias_sb[:, et:et + 1])

                    for dc in range(NDC):
                        # et order (2,3,0,1): the next pass's first matmuls
                        # then wait on an acc bank (freed by one [128,512]
                        # evac op) instead of a 1024-wide sc bank
                        for et in (2, 3, 0, 1):
                            lhsT = wsrc[:, dc, et * P:(et + 1) * P]
                            for i in range(2):
                                sb_ = 2 * sbp + i
                                out = (psA[et][:, i * 512:(i + 1) * 512]
                                       if et < 2 else psB[et][i][:])
                                nc.tensor.matmul(
                                    out, lhsT,
                                    xt_t[dc][:, sb_ * 512:(sb_ + 1) * 512],
                                    start=(dc == 0), stop=(dc == NDC - 1))
                            if dc == NDC - 1:
                                # evac emitted right behind this e-tile's
                                # final matmul: its engine starts draining
                                # the bank ~0.7us sooner, shrinking the
                                # next pass's PSUM-wait at the boundary
                                evac_et(et)

                qk_pass(wq_sb, qt_t, bq_sb, 0)
                qk_pass(wq_sb, qt_t, bq_sb, 1)

            # wq pool closed; K repeats the same pass shape from wk_sb
            for sbp in range(2):
                psA = {et: pst.tile([P, 1024], F32, name=f"kpsA{et}",
                                    tag="sc", bufs=2) for et in (0, 1)}
                psB = {et: [pst.tile([P, 512], F32, name=f"kpsB{et}_{i}",
                                     tag="acc", bufs=4) for i in range(2)]
                       for et in (2, 3)}
                def evac_ket(et):
                    use_act = (et % 2 == 1)
                    srcs = ([psA[et][:]] if et < 2
                            else [psB[et][0][:], psB[et][1][:]])
                    off = sbp * 1024
                    for src in srcs:
                        w = src.shape[-1]
                        dst = kt_t[et][:, off:off + w]
                        off += w
                        if use_act:
                            nc.scalar.activation(
                                dst, src,
                                mybir.ActivationFunctionType.Identity,
                                bias=bk_sb[:, et:et + 1])
                        else:
                            nc.vector.tensor_scalar_add(
                                dst, src, bk_sb[:, et:et + 1])

                for dc in range(NDC):
                    for et in (2, 3, 0, 1):
                        lhsT = wk_sb[:, dc, et * P:(et + 1) * P]
                        for i in range(2):
                            sb_ = 2 * sbp + i
                            out = (psA[et][:, i * 512:(i + 1) * 512]
                                   if et < 2 else psB[et][i][:])
                            nc.tensor.matmul(
                                out, lhsT,
                                xt_t[dc][:, sb_ * 512:(sb_ + 1) * 512],
                                start=(dc == 0), stop=(dc == NDC - 1))
                        if dc == NDC - 1:
                            evac_ket(et)

            # i-block 3 scores + exps: their ACT work overlaps the V
            # matmuls, taking ~18us of exp off the phase-2 critical path
            group_etiles[(3, 0)] = sc_group(3, 0, lambda jt: e30_t[jt])
            group_etiles[(3, 1)] = sc_group(3, 1, lambda jt: e31_t[jt],
                                            jt_lo=0, jt_hi=6)

            # V: out[s_tile(128), e(512)] accumulated over d-chunks
            for st in range(NST):
                psv = pst.tile([P, 512], F32, name="psv", tag="acc", bufs=4)
                for dc in range(NDC):
                    nc.tensor.matmul(
                        psv[:], xt_t[dc][:, st * P:(st + 1) * P],
                        wv_sb[:, dc, :],
                        start=(dc == 0), stop=(dc == NDC - 1))
                if st % 2 == 0:
                    nc.vector.tensor_copy(
                        v_t[st][:, :, 0:DK],
                        psv[:].rearrange("p (h w) -> p h w", h=HPC))
                else:
                    nc.scalar.activation(
                        v_t[st][:, :, 0:DK],
                        psv[:].rearrange("p (h w) -> p h w", h=HPC),
                        mybir.ActivationFunctionType.Copy)

        # ------------------------------------------------------------------
        # Phase 2+3, software-pipelined one group deep, i-blocks DESCENDING
        # ------------------------------------------------------------------
        with tc.tile_pool(name="p2", bufs=1) as p2, \
                tc.tile_pool(name="exps", bufs=26) as epool, \
                tc.tile_pool(name="small", bufs=8) as spool, \
                tc.tile_pool(name="yout", bufs=3) as ypool:
            wot_sb = p2.tile([P, HPC, D], BF, name="wot_sb", tag="wot_sb")
            nc.sync.dma_start(
                wot_sb[:], wot.rearrange("(ec p) d -> p ec d", p=P))

            # o tiles: 4 s-tiles per i-block, ring of 8 (two i-blocks in
            # flight); ot (transposed O) tiles: 4 per s-tile, ring of 8.
            o_tiles = {}   # ib -> [4 tiles]

            def av_one(ib, h, k, t, etiles, par):
                it = 4 * ib + t
                po = pst.tile([P, 512], F32, name="po", tag="acc", bufs=4)
                for jt in range(it + 1):
                    nc.tensor.matmul(
                        po[:, 0:DKP],
                        etiles[jt][:, k * 512 + t * P:k * 512 + (t + 1) * P],
                        v_t[jt][:, h, :],
                        start=(jt == 0), stop=(jt == it))
                if ib <= 1:
                    # HAM keep-warm: the last two i-blocks' AV groups are
                    # small matmuls threaded between evac chains -- PE
                    # activity drops enough for the HAM to halve the clock.
                    # A dependency-free dummy matmul into the UNUSED upper
                    # columns of this po bank fills the chain gaps (no
                    # extra PSUM slot, subtile-disjoint from cols 0:129).
                    nc.tensor.matmul(po[:, 384:512], wupd[:],
                                     wupd[:], start=True, stop=True)
                rec = spool.tile([P, 1], F32, name="rec", tag="rec")
                nc.vector.reciprocal(rec[:], po[:, DK:DKP])
                od = o_tiles[ib][t][:, h * P:(h + 1) * P]
                # o-evac stays on DVE: ACT must keep ahead on exps (score
                # matmuls block on ACT's PSUM reads to reuse the sc ring,
                # and exps run until the very last slot)
                nc.vector.tensor_scalar_mul(od, po[:, 0:DK], rec[:])

            def tp_p3_one(ib, t):
                st = 4 * ib + t
                o_st = o_tiles[ib][t]
                ots = []
                for ec in range(HPC):
                    # transpose via regular matmul against the identity
                    # (pipelines at stream rate, unlike PE transpose-mode)
                    pt = pst.tile([P, P], F32, name="pt", tag="acc", bufs=4)
                    nc.tensor.matmul(
                        pt[:], o_st[:, ec * P:(ec + 1) * P], ident[:],
                        start=True, stop=True)
                    ot = spool.tile([P, P], BF, name="ot", tag="ot", bufs=8)
                    nc.vector.tensor_copy(ot[:], pt[:])
                    ots.append(ot)
                y = ypool.tile([P, S], BF, name="y", tag="y")
                # the LAST-EMITTED s-tile is ib0/t3 (descending i-block
                # order), not st15 -- its DMA is what gates the kernel end
                last = (ib == 0 and t == 3)
                for ob in range(NSB):
                    py = pst.tile([P, 512], F32, name="py", tag="acc", bufs=4)
                    for ec in range(HPC):
                        nc.tensor.matmul(
                            py[:], ots[ec][:],
                            wot_sb[:, ec, ob * 512:(ob + 1) * 512],
                            start=(ec == 0), stop=(ec == HPC - 1))
                    # y casts alternate DVE/ACT: the ~690ns PSUM->bf16 casts
                    # gate the shared acc-PSUM ring
                    if last and ob == 3:
                        # final quarter's cast split across BOTH engines:
                        # it gates the kernel-ending DMA
                        nc.vector.tensor_copy(
                            y[:, ob * 512:ob * 512 + 256], py[:, 0:256])
                        nc.scalar.activation(
                            y[:, ob * 512 + 256:(ob + 1) * 512],
                            py[:, 256:512],
                            mybir.ActivationFunctionType.Copy)
                    elif ob % 2 == 0:
                        nc.vector.tensor_copy(
                            y[:, ob * 512:(ob + 1) * 512], py[:])
                    else:
                        nc.scalar.activation(
                            y[:, ob * 512:(ob + 1) * 512], py[:],
                            mybir.ActivationFunctionType.Copy)
                    # the last s-tile's output leaves in pieces so the
                    # final DMA doesn't serialize behind all 4 casts
                    if last and ob >= 1:
                        c0 = 0 if ob == 1 else ob * 512
                        nc.sync.dma_start(
                            outd[st * P:(st + 1) * P, c0:(ob + 1) * 512],
                            y[:, c0:(ob + 1) * 512])
                if not last:
                    nc.sync.dma_start(outd[st * P:(st + 1) * P, :], y[:])

            def av_group(ib, hp):
                ets = group_etiles[(ib, hp)]
                if hp == 0:
                    o_tiles[ib] = [
                        p2.tile([P, HPC * DK], BF, name=f"o{ib}_{t}",
                                tag="o", bufs=8) for t in range(4)]
                    for k in range(2):
                        for t in range(4):
                            av_one(ib, 2 * hp + k, k, t, ets,
                                   par=((k + t) % 2 == 0))
                else:
                    # transposes trail their s-tile's AVs by one t: the
                    # transpose LDW needs the evac of its s-tile's last
                    # AV; the lag gives that chain slack and spreads the
                    # evacuation load
                    for t in range(4):
                        for k in range(2):
                            av_one(ib, 2 * hp + k, k, t, ets,
                                   par=((k + t) % 2 == 1))
                        if t >= 1:
                            tp_p3_one(ib, t - 1)
                    tp_p3_one(ib, 3)

            def av_group_merged(ib):
                """Both head-pairs h-interleaved per s-tile: used for the
                LAST i-block, where the AV groups are thinnest -- 4 heads
                of matmul filler per s-tile (instead of 2) covers the
                o-evac -> transpose -> out-proj chain latency."""
                o_tiles[ib] = [
                    p2.tile([P, HPC * DK], BF, name=f"om{ib}_{t}",
                            tag="o", bufs=8) for t in range(4)]
                for t in range(4):
                    for h in range(HPC):
                        av_one(ib, h, h % 2, t, group_etiles[(ib, h // 2)],
                               par=(h % 2 == 1))
                    if t >= 1:
                        tp_p3_one(ib, t - 1)
                tp_p3_one(ib, 3)

            def et_ring(jt):
                return epool.tile([P, 1024], BF, name="et", tag="exp")

            # slot g emits scores+exp for group g, then AV/P3 for group g-1:
            # the scheduler uses the prior group's (fully-exp'd) work as PE
            # filler while this group's scores wait on ACT.  Group (3,0)
            # and half of (3,1) had scores+exps in P1, so their AV work
            # opens the reservoir.
            group_etiles[(3, 1)] = group_etiles[(3, 1)] + sc_group(
                3, 1, et_ring, jt_lo=6)
            av_group(3, 0)
            slots = [(2, 0), (2, 1), (1, 0)]
            prevs = [(3, 1), (2, 0), (2, 1)]
            for g, pg in zip(slots, prevs):
                group_etiles[g] = sc_group(g[0], g[1], et_ring)
                av_group(*pg)
            # the last two i-blocks' AV groups are thin (1-2us of matmuls):
            # run them h-merged, with their sc groups emitted just ahead --
            # the scheduler drains the merged AVs as the exps flow in
            group_etiles[(1, 1)] = sc_group(1, 1, et_ring)
            group_etiles[(0, 0)] = sc_group(0, 0, et_ring)
            av_group_merged(1)
            group_etiles[(0, 1)] = sc_group(0, 1, et_ring)
            av_group_merged(0)

    nc.finalize()
    return nc


_NC_CACHE = {}


def _get_nc():
    if "nc" not in _NC_CACHE:
        _NC_CACHE["nc"] = _build_nc()
    return _NC_CACHE["nc"]


def _make_in_maps(x, wq, bq, wk, bk, wv, wo):
    scale = np.float32(1.0 / np.sqrt(DK))
    in_maps = []
    for c in range(NCORES):
        b = c // 4
        g = c % 4
        sl = slice(E * g, E * (g + 1))
        wqt = (wq[sl] * scale).T
        in_maps.append({
            "xt": np.ascontiguousarray(x[b].T).astype(BF16),
            "wqt": np.ascontiguousarray(wqt).astype(BF16),
            "wkt": np.ascontiguousarray(wk[sl].T).astype(BF16),
            "wvt": np.ascontiguousarray(wv[sl].T).astype(BF16),
            "wot": np.ascontiguousarray(wo[:, sl].T).astype(BF16),
            "bq": np.ascontiguousarray(
                (bq[sl] * scale).reshape(HPC, P).T).astype(np.float32),
            "bk": np.ascontiguousarray(
                bk[sl].reshape(HPC, P).T).astype(np.float32),
        })
    return in_maps


def _assemble(core_outs, wv_bias_vec):
    out = np.empty((B, S, D), np.float32)
    for b in range(B):
        acc = core_outs[4 * b].astype(np.float32)
        for g in range(1, 4):
            acc = acc + core_outs[4 * b + g].astype(np.float32)
        out[b] = acc + wv_bias_vec
    return out


def kernel(x, wq, bq, wk, bk, wv, bv, wo, bo, mask, _trace=False):
    x = np.asarray(x, dtype=np.float32)
    wq = np.asarray(wq, dtype=np.float32)
    bq = np.asarray(bq, dtype=np.float32)
    wk = np.asarray(wk, dtype=np.float32)
    bk = np.asarray(bk, dtype=np.float32)
    wv = np.asarray(wv, dtype=np.float32)
    bv = np.asarray(bv, dtype=np.float32)
    wo = np.asarray(wo, dtype=np.float32)
    bo = np.asarray(bo, dtype=np.float32)

    in_maps = _make_in_maps(x, wq, bq, wk, bk, wv, wo)
    nc = _get_nc()
    res = run_bass_kernel_spmd(nc, in_maps, core_ids=list(range(NCORES)),
                               trace=_trace)
    core_outs = [res.results[c]["out"] for c in range(NCORES)]
    # rows of softmax sum to 1 -> per-head V-bias contributes wo[:, sl] @ bv
    bias_vec = (bo + wo @ bv).astype(np.float32)
    out = _assemble(core_outs, bias_vec)
    if _trace:
        return out, res
    return out
